# revision 1
# baseline (speedup 1.0000x reference)
"""Trainium2 Bass kernel for nn_Attention_46918222741521 (SAN pairwise attention).

Computation (per batch n):
  q = W1@x, k = W2@x, v = W3@x (1x1 convs), p = positional encoding (2ch)
  t[c,p,yx]   = concat( q[c,yx] - unfold7x7(k)[c,p,yx],  p_center - unfold(p) )
  u = relu(bn1(t)); z = cw1@u; r = relu(bn2(z)); logits = cw2@r (+b, cancels)
  wts = softmax_p(logits);  out[c,yx] = sum_p wts[g(c),p,yx]*unfold(v)[c,p,yx]
  g(c) = c // 8  (share_planes=8)

Sharding: 8 cores = 4 batches x 2 H-halves (28 rows each + 3-row reflect halo).

Device pipeline per core (SPMD, one program, per-core data):
  PE:  q [4pg x 32, 7*56] pixgroup-packed via col-tiling; k [4pg x 32, 13*62];
       v s-major-permuted [2 x 128, 34*62] (host-permuted W3 columns).
  Per (row-set s in 0..6, x-half) over rows {s, 7+s, 14+s, 21+s}:
       u = relu(q - k_shift)                (DVE, bf16, j-parity split for 2x)
       z = CW1m @ u                         (PE 4-tile diagonal, K=M=32)
       r = relu(z + POS_T[s])               (DVE add + ACT relu; table from host)
       logits_rep[rho] = CW2R @ r           (PE col-replicated M=128 so e lands
                                             4x-replicated across partition blocks)
       e = exp(logits_rep)                  (ACT, PSUM->SBUF bf16)
       Z = reduce_p(e) (gpsimd), rz = 1/Z  (DVE)
       prod = v_shift * e                   (DVE TT bf16 2x, j-parity split)
       out = treeadd_p(prod) * rz           (DVE; bf16 first levels, f32 tail)
       DMA out rows to DRAM (AP unpermutes s-major -> natural channel order).

BN folded into weights on host; cw2 bias dropped (softmax-invariant); positional
branch precomputed on host into per-row-set additive tables POS_T (it depends
only on weights, never on x).
"""

import sys
import numpy as np

sys.path.insert(0, "/opt/trn_rl_repo")

KS = 7
PAD = 3
BN_EPS = 1e-5
N, CIN, H, W = 4, 256, 56, 56
REL = 32
G = 32
S = 8
ROWS = 28          # output rows per core
HROWS = ROWS + 6   # 34 input rows per core
WPAD = W + 6       # 62
NSETS = 7
NRHO = 4
XH = 28            # x-half width
FS = KS * KS * XH  # 1372 free elems per (p, x-half)
KF = 13 * WPAD     # 806: k free elems per pixgroup (13 halo rows x 62)
QF = 7 * W         # 392: q free elems (7 rows x 56)
VF = HROWS * WPAD  # 2108

PERM_SM = (8 * (np.arange(256) % 32) + (np.arange(256) // 32)).astype(np.int64)


def _np(x):
    return np.asarray(x)


def _fold_bn(g, b, m, v):
    s = g / np.sqrt(v + BN_EPS)
    return s.astype(np.float64), (b - m * s).astype(np.float64)


def _pos_p(conv_p_w, conv_p_b):
    loc_w = np.broadcast_to(np.linspace(-1.0, 1.0, W)[None, :], (H, W))
    loc_h = np.broadcast_to(np.linspace(-1.0, 1.0, H)[:, None], (H, W))
    loc = np.stack([loc_w, loc_h], 0)
    return np.einsum('oc,chw->ohw', conv_p_w.astype(np.float64), loc) \
        + conv_p_b.astype(np.float64).reshape(2, 1, 1)


def _unfold(x):
    xp = np.pad(x, ((0, 0), (PAD, PAD), (PAD, PAD)), mode='reflect')
    C = x.shape[0]
    out = np.empty((C, KS * KS, x.shape[1], x.shape[2]), xp.dtype)
    for i in range(KS):
        for j in range(KS):
            out[:, i * KS + j] = xp[:, i:i + x.shape[1], j:j + x.shape[2]]
    return out


def host_prep(w1, w2, w3, conv_p_w, conv_p_b, bn1_g, bn1_b, bn1_m, bn1_v,
              cw1_w, bn2_g, bn2_b, bn2_m, bn2_v, cw2_w, cw2_b):
    s1, b1 = _fold_bn(bn1_g, bn1_b, bn1_m, bn1_v)
    s2, b2 = _fold_bn(bn2_g, bn2_b, bn2_m, bn2_v)

    W1 = w1.astype(np.float64) * s1[:REL, None]
    b1q = b1[:REL]
    W2 = w2.astype(np.float64) * s1[:REL, None]
    W3P = w3.astype(np.float64)[PERM_SM]
    CW1m = cw1_w.astype(np.float64)[:, :REL] * s2[:, None]

    p = _pos_p(conv_p_w, conv_p_b)
    subp = p[:, None] - _unfold(p)
    u_pos = np.maximum(
        s1[REL:, None, None, None] * subp + b1[REL:, None, None, None], 0.0)
    pos_zb = np.einsum('oc,cphw->ophw',
                       cw1_w.astype(np.float64)[:, REL:] * s2[:, None], u_pos) \
        + b2[:, None, None, None]

    CW2 = cw2_w.astype(np.float64)
    return dict(W1=W1, b1q=b1q, W2=W2, W3P=W3P, CW1m=CW1m, POS_ZB=pos_zb,
                CW2=CW2)


def shard_x(x, core):
    n, half = core // 2, core % 2
    xp = np.pad(x[n], ((0, 0), (PAD, PAD), (PAD, PAD)), mode='reflect')
    r0 = ROWS * half
    return xp[:, r0:r0 + HROWS, :]


# ---------------------------------------------------------------------------
# numpy model of the device pipeline (for validation)
# ---------------------------------------------------------------------------

def numpy_model_core(x_halo, hp, core):
    half = core % 2
    r0 = ROWS * half
    xh = x_halo.astype(np.float64).reshape(CIN, -1)

    q = (hp['W1'] @ xh).reshape(REL, HROWS, WPAD)[:, PAD:PAD + ROWS, PAD:PAD + W] \
        + hp['b1q'][:, None, None]
    k = (hp['W2'] @ xh).reshape(REL, HROWS, WPAD)
    v = (hp['W3P'] @ xh).reshape(256, HROWS, WPAD)

    out_sm = np.zeros((256, ROWS, W))
    for s in range(NSETS):
        for rho in range(NRHO):
            lr = 7 * rho + s
            gy = r0 + lr
            u = np.empty((REL, KS * KS, W))
            for i in range(KS):
                for j in range(KS):
                    u[:, i * KS + j] = q[:, lr] - k[:, lr + i, j:j + W]
            u = np.maximum(u, 0.0)
            z = np.einsum('oc,cpx->opx', hp['CW1m'], u)
            r = np.maximum(z + hp['POS_ZB'][:, :, gy, :], 0.0)
            e = np.exp(np.einsum('go,opx->gpx', hp['CW2'], r))
            Z = e.sum(axis=1)
            for sv in range(S):
                for i in range(KS):
                    for j in range(KS):
                        out_sm[sv * 32:(sv + 1) * 32, lr] += \
                            e[:, i * KS + j] * v[sv * 32:(sv + 1) * 32, lr + i, j:j + W]
                out_sm[sv * 32:(sv + 1) * 32, lr] /= Z
    return out_sm[np.argsort(PERM_SM)]


def kernel_numpy(**inputs):
    hp = host_prep(**{k: _np(v) for k, v in inputs.items() if k != 'x'})
    x = _np(inputs['x'])
    out = np.zeros((N, 256, H, W))
    for core in range(8):
        n, half = core // 2, core % 2
        out[n, :, ROWS * half:ROWS * (half + 1), :] = \
            numpy_model_core(shard_x(x, core), hp, core)
    return out.astype(np.float32)


# ---------------------------------------------------------------------------
# Bass kernel
# ---------------------------------------------------------------------------

def _ap(t, base, dims, pbase=0, pcount=128):
    """Strided free-dim view of an SBUF tile AP. dims: [[step,count],...]."""
    import concourse.bass as bass
    pitch = t.ap[0][0]
    return bass.AP(tensor=t.tensor,
                   offset=t.offset + pbase * pitch + base,
                   ap=[[pitch, pcount]] + [list(d) for d in dims])


def _dram_ap(handle_ap, base, dims):
    import concourse.bass as bass
    return bass.AP(tensor=handle_ap.tensor, offset=handle_ap.offset + base,
                   ap=[list(d) for d in dims])


def build_nc(trace_sim=False):
    import concourse.bass as bass
    import concourse.bacc as bacc
    import concourse.tile as tile
    from concourse import mybir
    from contextlib import ExitStack

    BF = mybir.dt.bfloat16
    F32 = mybir.dt.float32
    Alu = mybir.AluOpType
    Act = mybir.ActivationFunctionType
    Axis = mybir.AxisListType

    nc = bacc.Bacc("TRN2", target_bir_lowering=False, debug=False,
                   num_devices=8)

    xh_d = nc.dram_tensor("xh", [CIN, VF], BF, kind="ExternalInput").ap()
    w1T_d = nc.dram_tensor("w1T", [CIN, REL], BF, kind="ExternalInput").ap()
    w2T_d = nc.dram_tensor("w2T", [CIN, REL], BF, kind="ExternalInput").ap()
    w3T_d = nc.dram_tensor("w3T", [CIN, 256], BF, kind="ExternalInput").ap()
    cw1R_d = nc.dram_tensor("cw1R", [128, 32], BF, kind="ExternalInput").ap()
    cw2R_d = nc.dram_tensor("cw2R", [128, 128], BF, kind="ExternalInput").ap()
    b1q_d = nc.dram_tensor("b1q", [128, 1], F32, kind="ExternalInput").ap()
    id32_d = nc.dram_tensor("id32", [128, 32], BF, kind="ExternalInput").ap()
    pos_d = nc.dram_tensor("posT", [NSETS, 2, 128, FS], BF,
                           kind="ExternalInput").ap()
    out_d = nc.dram_tensor("out", [256, ROWS * W], F32,
                           kind="ExternalOutput").ap()

    with tile.TileContext(nc, trace_sim=trace_sim) as tc, ExitStack() as ctx:
        singles = ctx.enter_context(tc.tile_pool(name="singles", bufs=1))

        # ---- resident SBUF tensors ----
        xh_sb = []
        for h in range(2):
            t = singles.tile([128, VF], BF, tag=f"xh{h}", name=f"xh{h}")
            nc.sync.dma_start(out=t, in_=xh_d[128 * h:128 * (h + 1), :])
            xh_sb.append(t)
        w1T_sb, w2T_sb = [], []
        for h in range(2):
            t = singles.tile([128, REL], BF, tag=f"w1T{h}", name=f"w1T{h}")
            nc.sync.dma_start(out=t, in_=w1T_d[128 * h:128 * (h + 1), :])
            w1T_sb.append(t)
            t = singles.tile([128, REL], BF, tag=f"w2T{h}", name=f"w2T{h}")
            nc.sync.dma_start(out=t, in_=w2T_d[128 * h:128 * (h + 1), :])
            w2T_sb.append(t)
        w3T_sb = {}
        for kk in range(2):
            for mh in range(2):
                t = singles.tile([128, 128], BF, tag=f"w3T{kk}{mh}", name=f"w3T{kk}{mh}")
                nc.sync.dma_start(
                    out=t, in_=w3T_d[128 * kk:128 * (kk + 1),
                                     128 * mh:128 * (mh + 1)])
                w3T_sb[(kk, mh)] = t
        cw1R_sb = singles.tile([128, 32], BF, tag="cw1R", name="cw1R")
        nc.sync.dma_start(out=cw1R_sb, in_=cw1R_d)
        cw2R_sb = singles.tile([128, 128], BF, tag="cw2R", name="cw2R")
        nc.sync.dma_start(out=cw2R_sb, in_=cw2R_d)
        b1q_sb = singles.tile([128, 1], F32, tag="b1q", name="b1q")
        nc.sync.dma_start(out=b1q_sb, in_=b1q_d)
        id32_sb = singles.tile([128, 32], BF, tag="id32", name="id32")
        nc.sync.dma_start(out=id32_sb, in_=id32_d)

        q_sb = singles.tile([128, QF], BF, tag="q", name="q")
        k_sb = singles.tile([128, KF], BF, tag="k", name="k")
        k_od = singles.tile([128, KF - 2], BF, tag="k_od", name="k_od")
        v_sb = [singles.tile([128, VF], BF, tag=f"v{h}", name=f"v{h}") for h in range(2)]
        v_od = [singles.tile([128, VF - 2], BF, tag=f"v_od{h}", name=f"v_od{h}")
                for h in range(2)]

        # ---- phase A: q, k, v projections ----
        with tc.tile_pool(name="psA", bufs=1, space="PSUM") as psA:
            q_ps = psA.tile([128, QF], F32, tag="qp", name="qp")
            for a in range(4):
                for kk in range(2):
                    nc.tensor.matmul(
                        q_ps[32 * a:32 * (a + 1), :],
                        lhsT=w1T_sb[kk],
                        rhs=_ap(xh_sb[kk], (7 * a + PAD) * WPAD + PAD,
                                [[WPAD, 7], [1, W]]),
                        start=(kk == 0), stop=(kk == 1),
                        tile_position=(0, 32 * a))
            nc.scalar.activation(q_sb[:, :], q_ps[:, :], Act.Identity,
                                 bias=b1q_sb[:, :], scale=1.0)

            k_ps = psA.tile([128, KF], F32, tag="kp", name="kp")
            for a in range(4):
                for kk in range(2):
                    for c0, cn in [(0, 512), (512, KF - 512)]:
                        nc.tensor.matmul(
                            k_ps[32 * a:32 * (a + 1), c0:c0 + cn],
                            lhsT=w2T_sb[kk],
                            rhs=_ap(xh_sb[kk], 7 * a * WPAD + c0, [[1, cn]]),
                            start=(kk == 0), stop=(kk == 1),
                            tile_position=(0, 32 * a))
            nc.scalar.copy(k_sb[:, :], k_ps[:, :])

            vchunks = [(i * 512, min(512, VF - i * 512))
                       for i in range((VF + 511) // 512)]
            for mh in range(2):
                v_ps = psA.tile([128, VF], F32, tag="vp", name="vp")
                for kk in range(2):
                    for c0, cn in vchunks:
                        nc.tensor.matmul(
                            v_ps[:, c0:c0 + cn],
                            lhsT=w3T_sb[(kk, mh)],
                            rhs=xh_sb[kk][:, c0:c0 + cn],
                            start=(kk == 0), stop=(kk == 1))
                nc.scalar.copy(v_sb[mh][:, :], v_ps[:, :])
        nc.gpsimd.tensor_copy(k_od[:, :], k_sb[:, 1:KF - 1])
        for h in range(2):
            nc.gpsimd.tensor_copy(v_od[h][:, :], v_sb[h][:, 1:VF - 1])

        # ---- phase B: per (row-set, x-half) pipeline ----
        pos_pool = ctx.enter_context(tc.tile_pool(name="pos", bufs=2))
        work = ctx.enter_context(tc.tile_pool(name="work", bufs=2))
        epool = ctx.enter_context(tc.tile_pool(name="e", bufs=2))
        tpool = ctx.enter_context(tc.tile_pool(name="tree", bufs=2))
        opool = ctx.enter_context(tc.tile_pool(name="outp", bufs=3))
        psB = ctx.enter_context(tc.tile_pool(name="psB", bufs=2, space="PSUM"))

        for s in range(NSETS):
            for xh in range(2):
                xb = xh * XH
                pos_sb = pos_pool.tile([128, FS], BF, tag="pos", name="pos")
                nc.sync.dma_start(out=pos_sb, in_=pos_d[s, xh])

                # u = relu(q - k_shift)   [128 = 4pg x 32, 49*28] bf16
                u_sb = work.tile([128, FS], BF, tag="u", name="u")
                qv_e = _ap(q_sb, s * W + xb, [[0, 7], [0, 4], [1, XH]])
                qv_o = _ap(q_sb, s * W + xb, [[0, 7], [0, 3], [1, XH]])
                nc.vector.tensor_tensor(
                    _ap(u_sb, 0, [[7 * XH, 7], [2 * XH, 4], [1, XH]]),
                    qv_e,
                    _ap(k_sb, s * WPAD + xb, [[WPAD, 7], [2, 4], [1, XH]]),
                    Alu.subtract)
                nc.vector.tensor_tensor(
                    _ap(u_sb, XH, [[7 * XH, 7], [2 * XH, 3], [1, XH]]),
                    qv_o,
                    _ap(k_od, s * WPAD + xb, [[WPAD, 7], [2, 3], [1, XH]]),
                    Alu.subtract)
                nc.vector.tensor_scalar_max(u_sb[:, :], u_sb[:, :], 0.0)

                # z = CW1m @ u  (4 diagonal tiles)
                z_ps = psB.tile([128, FS], F32, tag="zlg", name="zlg")
                for a in range(4):
                    for c0, cn in [(0, 512), (512, 512), (1024, FS - 1024)]:
                        nc.tensor.matmul(
                            z_ps[32 * a:32 * (a + 1), c0:c0 + cn],
                            lhsT=cw1R_sb[32 * a:32 * (a + 1), :],
                            rhs=u_sb[32 * a:32 * (a + 1), c0:c0 + cn],
                            start=True, stop=False,
                            tile_position=(32 * a, 32 * a))
                        nc.tensor.matmul(
                            z_ps[32 * a:32 * (a + 1), c0:c0 + cn],
                            lhsT=id32_sb[32 * a:32 * (a + 1), :],
                            rhs=pos_sb[32 * a:32 * (a + 1), c0:c0 + cn],
                            start=False, stop=True,
                            tile_position=(32 * a, 32 * a))

                # r = relu(z + pos): pos accumulated in PSUM; ACT relu-evicts
                r_sb = work.tile([128, FS], BF, tag="r", name="r")
                nc.scalar.activation(r_sb[:, :], z_ps[:, :], Act.Relu)

                # logits (4x replicated) + exp, per rho
                e_sb = epool.tile([128, NRHO, FS], BF, tag="e", name="e")
                for a in range(4):
                    lg_ps = psB.tile([128, FS], F32, tag="zlg", name="zlg")
                    for c0, cn in [(0, 512), (512, 512),
                                   (1024, FS - 1024)]:
                        nc.tensor.matmul(
                            lg_ps[:, c0:c0 + cn],
                            lhsT=cw2R_sb[32 * a:32 * (a + 1), :],
                            rhs=r_sb[32 * a:32 * (a + 1), c0:c0 + cn],
                            start=True, stop=True,
                            tile_position=(32 * a, 0))
                    nc.scalar.activation(e_sb[:, a, :], lg_ps[:, :], Act.Exp)

                # Z = sum_p e (gpsimd), rz = 1/Z
                zt1 = tpool.tile([128, NRHO, 24, XH], BF, tag="zt1",
                                 name="zt1")
                nc.gpsimd.tensor_tensor(
                    zt1[:, :, :, :],
                    _ap(e_sb, 0, [[FS, 4], [2 * XH, 24], [1, XH]]),
                    _ap(e_sb, XH, [[FS, 4], [2 * XH, 24], [1, XH]]),
                    Alu.add)
                zt2 = tpool.tile([128, NRHO, 12, XH], BF, tag="zt2",
                                 name="zt2")
                nc.gpsimd.tensor_tensor(
                    zt2[:, :, :, :],
                    _ap(zt1, 0, [[24 * XH, 4], [2 * XH, 12], [1, XH]]),
                    _ap(zt1, XH, [[24 * XH, 4], [2 * XH, 12], [1, XH]]),
                    Alu.add)
                zt3 = tpool.tile([128, NRHO, 6, XH], F32, tag="zt3",
                                 name="zt3")
                nc.gpsimd.tensor_tensor(
                    zt3[:, :, :, :],
                    _ap(zt2, 0, [[12 * XH, 4], [2 * XH, 6], [1, XH]]),
                    _ap(zt2, XH, [[12 * XH, 4], [2 * XH, 6], [1, XH]]),
                    Alu.add)
                zt4 = tpool.tile([128, NRHO, 3, XH], F32, tag="zt4",
                                 name="zt4")
                nc.gpsimd.tensor_tensor(
                    zt4[:, :, :, :],
                    _ap(zt3, 0, [[6 * XH, 4], [2 * XH, 3], [1, XH]]),
                    _ap(zt3, XH, [[6 * XH, 4], [2 * XH, 3], [1, XH]]),
                    Alu.add)
                zs = opool.tile([128, NRHO, XH], F32, tag="Z", name="Z")
                nc.vector.tensor_tensor(
                    zs[:, :, :], zt4[:, :, 0, :], zt4[:, :, 1, :], Alu.add)
                nc.vector.tensor_tensor(
                    zs[:, :, :], zs[:, :, :], zt4[:, :, 2, :], Alu.add)
                nc.vector.tensor_tensor(
                    zs[:, :, :], zs[:, :, :],
                    _ap(e_sb, 48 * XH, [[FS, 4], [1, XH]]), Alu.add)
                rz = opool.tile([128, NRHO, XH], F32, tag="rz", name="rz")
                nc.vector.reciprocal(rz[:, :, :], zs[:, :, :])

                for h in range(2):
                    # prod = v_shift * e  (j-parity split, rho batched via AP)
                    prod = work.tile([128, NRHO, FS], BF, tag=f"prod{h}", name=f"prod{h}")
                    vbase = s * WPAD + xb
                    nc.vector.tensor_tensor(
                        _ap(prod, 0,
                            [[FS, 4], [7 * XH, 7], [2 * XH, 4], [1, XH]]),
                        _ap(v_sb[h], vbase,
                            [[7 * WPAD, 4], [WPAD, 7], [2, 4], [1, XH]]),
                        _ap(e_sb, 0,
                            [[FS, 4], [7 * XH, 7], [2 * XH, 4], [1, XH]]),
                        Alu.mult)
                    nc.vector.tensor_tensor(
                        _ap(prod, XH,
                            [[FS, 4], [7 * XH, 7], [2 * XH, 3], [1, XH]]),
                        _ap(v_od[h], vbase,
                            [[7 * WPAD, 4], [WPAD, 7], [2, 3], [1, XH]]),
                        _ap(e_sb, XH,
                            [[FS, 4], [7 * XH, 7], [2 * XH, 3], [1, XH]]),
                        Alu.mult)
                    # tree-add over p: 49 -> 24 -> 12 -> 6 -> 3 -> 1 (+carries)
                    t1 = tpool.tile([128, NRHO, 24, XH], BF, tag="t1", name="t1")
                    nc.gpsimd.tensor_tensor(
                        t1[:, :, :, :],
                        _ap(prod, 0, [[FS, 4], [2 * XH, 24], [1, XH]]),
                        _ap(prod, XH, [[FS, 4], [2 * XH, 24], [1, XH]]),
                        Alu.add)
                    t2 = tpool.tile([128, NRHO, 12, XH], BF, tag="t2", name="t2")
                    nc.gpsimd.tensor_tensor(
                        t2[:, :, :, :],
                        _ap(t1, 0, [[24 * XH, 4], [2 * XH, 12], [1, XH]]),
                        _ap(t1, XH, [[24 * XH, 4], [2 * XH, 12], [1, XH]]),
                        Alu.add)
                    t3 = tpool.tile([128, NRHO, 6, XH], BF, tag="t3", name="t3")
                    nc.vector.tensor_tensor(
                        t3[:, :, :, :],
                        _ap(t2, 0, [[12 * XH, 4], [2 * XH, 6], [1, XH]]),
                        _ap(t2, XH, [[12 * XH, 4], [2 * XH, 6], [1, XH]]),
                        Alu.add)
                    t4 = tpool.tile([128, NRHO, 3, XH], F32, tag="t4", name="t4")
                    nc.vector.tensor_tensor(
                        t4[:, :, :, :],
                        _ap(t3, 0, [[6 * XH, 4], [2 * XH, 3], [1, XH]]),
                        _ap(t3, XH, [[6 * XH, 4], [2 * XH, 3], [1, XH]]),
                        Alu.add)
                    t5 = tpool.tile([128, NRHO, XH], F32, tag="t5", name="t5")
                    nc.vector.tensor_tensor(
                        t5[:, :, :], t4[:, :, 0, :], t4[:, :, 1, :], Alu.add)
                    nc.vector.tensor_tensor(
                        t5[:, :, :], t5[:, :, :], t4[:, :, 2, :], Alu.add)
                    nc.vector.tensor_tensor(
                        t5[:, :, :], t5[:, :, :],
                        _ap(prod, 48 * XH, [[FS, 4], [1, XH]]), Alu.add)
                    osum = opool.tile([128, NRHO, XH], F32, tag=f"os{h}", name=f"os{h}")
                    nc.vector.tensor_tensor(
                        osum[:, :, :], t5[:, :, :], rz[:, :, :], Alu.mult)
                    # DMA out, unpermuting s-major -> natural channels
                    dst = _dram_ap(
                        out_d, (128 * h) * (ROWS * W) + s * W + xb,
                        [[ROWS * W, 128], [7 * W, 4], [1, XH]])
                    nc.sync.dma_start(out=dst, in_=osum[:, :, :])
    nc.finalize()
    return nc


_NC_CACHE = {}


def _get_nc():
    if "nc" not in _NC_CACHE:
        _NC_CACHE["nc"] = build_nc()
    return _NC_CACHE["nc"]


def make_in_maps(inputs):
    import ml_dtypes
    bf16 = ml_dtypes.bfloat16
    hp = host_prep(**{k: _np(v) for k, v in inputs.items() if k != 'x'})
    x = _np(inputs['x'])

    w1T = hp['W1'].T.astype(bf16)
    w2T = hp['W2'].T.astype(bf16)
    w3T = hp['W3P'].T.astype(bf16)
    cw1R = np.tile(hp['CW1m'].T, (4, 1)).astype(bf16)
    cw2R = np.tile(hp['CW2'].T, (4, 4)).astype(bf16)
    b1q = np.tile(hp['b1q'][:, None], (4, 1)).astype(np.float32)
    id32 = np.tile(np.eye(32), (4, 1)).astype(bf16)

    in_maps = []
    for core in range(8):
        half = core % 2
        r0 = ROWS * half
        xh = shard_x(x, core).reshape(CIN, VF).astype(bf16)
        posT = np.empty((NSETS, 2, 128, FS), np.float64)
        for s in range(NSETS):
            for a in range(4):
                blk = hp['POS_ZB'][:, :, r0 + 7 * a + s, :]  # [32, 49, 56]
                blk = blk.reshape(32, KS * KS, 2, XH)
                for xhh in range(2):
                    posT[s, xhh, 32 * a:32 * (a + 1), :] = \
                        blk[:, :, xhh, :].reshape(32, FS)
        in_maps.append(dict(
            xh=np.ascontiguousarray(xh),
            w1T=np.ascontiguousarray(w1T),
            w2T=np.ascontiguousarray(w2T),
            w3T=np.ascontiguousarray(w3T),
            cw1R=np.ascontiguousarray(cw1R),
            cw2R=np.ascontiguousarray(cw2R),
            b1q=np.ascontiguousarray(b1q),
            id32=np.ascontiguousarray(id32),
            posT=np.ascontiguousarray(posT.astype(bf16)),
        ))
    return in_maps


def _get_exec():
    """Build the sharded PJRT executable once and cache it."""
    if "exec" in _NC_CACHE:
        return _NC_CACHE["exec"]
    import jax
    from jax.sharding import Mesh, PartitionSpec, NamedSharding
    from jax.experimental.shard_map import shard_map
    from concourse import bass2jax, mybir
    from concourse.bass2jax import _bass_exec_p, install_neuronx_cc_hook

    install_neuronx_cc_hook()
    nc = _get_nc()
    pname = nc.partition_id_tensor.name if nc.partition_id_tensor else None
    in_names, out_names, out_avals, zero_outs = [], [], [], []
    for alloc in nc.m.functions[0].allocations:
        if not isinstance(alloc, mybir.MemoryLocationSet):
            continue
        name = alloc.memorylocations[0].name
        if alloc.kind == "ExternalInput":
            if name != pname:
                in_names.append(name)
        elif alloc.kind == "ExternalOutput":
            shape = tuple(alloc.tensor_shape)
            dtype = mybir.dt.np(alloc.dtype)
            out_names.append(name)
            out_avals.append(jax.core.ShapedArray(shape, dtype))
            zero_outs.append(np.zeros(shape, dtype))
    all_in = in_names + out_names + ([pname] if pname else [])

    def _body(*args):
        operands = list(args)
        if pname is not None:
            operands.append(bass2jax.partition_id_tensor())
        return tuple(_bass_exec_p.bind(
            *operands, out_avals=tuple(out_avals), in_names=tuple(all_in),
            out_names=tuple(out_names), lowering_input_output_aliases=(),
            sim_require_finite=True, sim_require_nnan=True, nc=nc))

    devices = jax.devices()[:8]
    mesh = Mesh(np.asarray(devices), ("core",))
    nin = len(in_names) + len(out_names)
    sharded = jax.jit(shard_map(_body, mesh=mesh,
                                in_specs=(PartitionSpec("core"),) * nin,
                                out_specs=(PartitionSpec("core"),) * len(out_names),
                                check_rep=False), keep_unused=True)
    shard = NamedSharding(mesh, PartitionSpec("core"))
    _NC_CACHE["exec"] = (sharded, shard, in_names, zero_outs)
    return _NC_CACHE["exec"]


def kernel(**inputs):
    in_maps = make_in_maps(inputs)
    out = np.zeros((N, 256, H, W), np.float32)
    try:
        import jax
        sharded, shard, in_names, zero_outs = _get_exec()
        concat = [np.concatenate([np.asarray(in_maps[c][nm])
                                  for c in range(8)], axis=0)
                  for nm in in_names]
        concat += [np.concatenate([z] * 8, axis=0) for z in zero_outs]
        dev_in = [jax.device_put(a, shard) for a in concat]
        outs = sharded(*dev_in)
        o = np.asarray(outs[0])
        res_per_core = [o[c * 256:(c + 1) * 256] for c in range(8)]
    except Exception:
        from concourse import bass_utils
        nc = _get_nc()
        res = bass_utils.run_bass_kernel_spmd(
            nc, in_maps, core_ids=list(range(8)))
        res_per_core = [res.results[c]["out"] for c in range(8)]
    inv = np.argsort(PERM_SM)
    for core in range(8):
        n, half = core // 2, core % 2
        out[n, :, ROWS * half:ROWS * (half + 1), :] = \
            res_per_core[core].reshape(256, ROWS, W)[inv]
    return out

